# revision 9
# baseline (speedup 1.0000x reference)
"""Trainium2 Bass kernel for ApertureChamberSSM (v2).

Computation (reference):
    iv, ov, beta_s, alpha, mg = sigmoid(scalars); decay = exp(-alpha)
    x_in  = iv * x ; drive = tanh(x_in)
    psi_s = decay * psi_{s-1} + (1-decay) * drive_s          (scan over S)
    x_mem = mg * psi + (1-mg) * x_in
    rotate channel pairs (j, j+512) by pi*sigmoid(beta), scale by ov

Algebra: psi = (1-decay)*psi' with psi'_s = decay*psi'_{s-1} + drive_s
    x_mem = ap_*psi' + c*x   (ap_ = mg*(1-decay), c = (1-mg)*iv)
    out_r = p*x_mem_r - q*x_mem_i ; out_i = q*x_mem_r + p*x_mem_i
          (p = cos(pi*beta_s)*ov, q = sin(pi*beta_s)*ov)
    => out_r = (p*ap_)psi_r + (p*c)x_r + (-q*ap_)psi_i + (-q*c)x_i
       out_i = (q*ap_)psi_r + (q*c)x_r + ( p*ap_)psi_i + ( p*c)x_i
    i.e. a 4-term combination done as accumulated scaled-identity matmuls
    on the TensorEngine (channels pairs live at the same partition index of
    the R-block and I-block tiles).

Engine assignment per core: DMA bf16 in/out; tanh on ACT (f32 drive);
tensor_tensor_scan on DVE (bf16 psi out); blend+rotation fused on PE as
4 accumulated bf16 matmuls per PSUM tile; PSUM->SBUF eviction on ACT
(bf16); host does sigmoid/cos/sin and the (de)sharding/transposes.

Sharding: core c owns channel pairs j in [64c, 64c+64) for all 4 batches:
shard (512, 8192): rows [0:256] real (b*64+j order), [256:512] imag.
"""

import math

import numpy as np

B, S, D = 4, 8192, 1024
HALF = D // 2          # 512
NCORES = 8
JPC = HALF // NCORES   # 64 channel pairs per core
ROWS = 2 * B * JPC     # 512 rows per core
P = 128                # partitions
C = 1024               # seq chunk (free dim) per tile
NCHUNK = S // C
NPAIR = ROWS // (2 * P)  # 2 block pairs: (R0,I0), (R1,I1)
MMF = 512              # matmul moving free dim (one PSUM bank)
SCG = 1024             # scan chaining granularity

_cache = {}


def _sig(v):
    return 1.0 / (1.0 + math.exp(-float(v)))


def _build(iv, decay, use_scan):
    """Build + compile the 8-core SPMD graph. Rotation/blend coefficients
    arrive at runtime via the 'consts' input (8 scaled identities), so only
    iv, decay and the use_scan flag are baked in."""
    import concourse.bass as bass
    import concourse.tile as tile
    from concourse import bacc, mybir

    f32 = mybir.dt.float32
    bf16 = mybir.dt.bfloat16
    AF = mybir.ActivationFunctionType
    OP = mybir.AluOpType

    nc = bacc.Bacc("TRN2", target_bir_lowering=False, debug=False,
                   num_devices=NCORES)
    x_ap = nc.dram_tensor("x", [ROWS, S], bf16, kind="ExternalInput").ap()
    consts_ap = nc.dram_tensor("consts", [P, 8 * P], bf16,
                               kind="ExternalInput").ap()
    out_ap = nc.dram_tensor("out", [ROWS, S], bf16, kind="ExternalOutput").ap()

    with tile.TileContext(nc) as tc:
        with (
            tc.tile_pool(name="const", bufs=1) as cpool,
            tc.tile_pool(name="xin", bufs=4) as xpool,
            tc.tile_pool(name="drv", bufs=3) as dpool,
            tc.tile_pool(name="psi", bufs=4) as ppool,
            tc.tile_pool(name="outs", bufs=4) as opool,
            tc.tile_pool(name="ps", bufs=2, space=bass.MemorySpace.PSUM) as pspool,
        ):
            idm = cpool.tile([P, 8 * P], bf16, tag="idm")
            nc.sync.dma_start(idm[:], consts_ap[:])
            # identity blocks: [p*ap_, p*c, -q*ap_, -q*c, q*ap_, q*c, pad, pad]
            lhs = [idm[:, j * P:(j + 1) * P] for j in range(8)]
            # coefficient order per psum group: [psi_r, x_r, psi_i, x_i]
            coef_r = [lhs[0], lhs[1], lhs[2], lhs[3]]
            coef_i = [lhs[4], lhs[5], lhs[0], lhs[1]]

            if use_scan:
                dk = cpool.tile([P, C], f32, tag="dk")
                nc.vector.memset(dk[:], decay)

            prev = [[None, None] for _ in range(NPAIR)]

            def front(k, i):
                xt, pt = [], []
                for h in range(2):  # 0 = real block, 1 = imag block
                    r0 = h * 2 * P + i * P
                    x_t = xpool.tile([P, C], bf16, tag=f"x{i}{h}")
                    nc.sync.dma_start(
                        x_t[:], x_ap[r0:r0 + P, k * C:(k + 1) * C])
                    xt.append(x_t)
                    if use_scan:
                        d_t = dpool.tile([P, C], f32, tag=f"d{i}{h}")
                        nc.scalar.activation(d_t[:], x_t[:], AF.Tanh,
                                             bias=0.0, scale=iv)
                        p_t = ppool.tile([P, C], bf16, tag=f"p{i}{h}")
                        for g in range(C // SCG):
                            gs = slice(g * SCG, (g + 1) * SCG)
                            init = (prev[i][h] if prev[i][h] is not None
                                    else 0.0)
                            nc.vector.tensor_tensor_scan(
                                p_t[:, gs], dk[:, gs], d_t[:, gs], init,
                                OP.mult, OP.add)
                            prev[i][h] = p_t[:, (g + 1) * SCG - 1:
                                             (g + 1) * SCG]
                        pt.append(p_t)
                return xt, pt

            def back(k, i, xt, pt):
                for h in range(2):
                    r0 = h * 2 * P + i * P
                    coef = coef_r if h == 0 else coef_i
                    if use_scan:
                        srcs = [pt[0], xt[0], pt[1], xt[1]]
                    else:
                        srcs = [xt[0], xt[1]]
                        coef = [coef[1], coef[3]]
                    o_t = opool.tile([P, C], bf16, tag=f"o{i}{h}")
                    for s4 in range(C // MMF):
                        fs = slice(s4 * MMF, (s4 + 1) * MMF)
                        ps = pspool.tile([P, MMF], f32, tag=f"ps{i}{h}")
                        for t in range(len(srcs)):
                            nc.tensor.matmul(
                                ps[:], coef[t], srcs[t][:, fs],
                                start=(t == 0), stop=(t == len(srcs) - 1))
                        nc.scalar.copy(o_t[:, fs], ps[:])
                    nc.sync.dma_start(
                        out_ap[r0:r0 + P, k * C:(k + 1) * C], o_t[:])

            pend = None
            for k in range(NCHUNK):
                for i in range(NPAIR):
                    cur = front(k, i)
                    if pend is not None:
                        back(*pend)
                    pend = (k, i, *cur)
            back(*pend)

    nc.compile()
    return nc


def kernel(x, beta, input_valve, output_valve, alpha_raw, memory_gate):
    x = np.asarray(x, dtype=np.float32)
    assert x.shape == (B, S, D), x.shape

    beta_s = _sig(beta)
    iv = _sig(input_valve)
    ov = _sig(output_valve)
    alpha = _sig(alpha_raw)
    mg = _sig(memory_gate)
    decay = math.exp(-alpha)
    c = (1.0 - mg) * iv
    ap_ = mg * (1.0 - decay)
    angle = math.pi * beta_s
    p_, q_ = math.cos(angle) * ov, math.sin(angle) * ov
    use_scan = ap_ != 0.0

    key = (round(iv, 12), round(decay, 12), use_scan)
    if key not in _cache:
        _cache[key] = _build(iv, decay, use_scan)
    nc = _cache[key]

    import ml_dtypes
    from concourse.bass_utils import run_bass_kernel_spmd

    bf = ml_dtypes.bfloat16
    eye = np.eye(P, dtype=np.float64)
    blocks = [p_ * ap_, p_ * c, -q_ * ap_, -q_ * c, q_ * ap_, q_ * c, 0.0, 0.0]
    consts = np.concatenate([b * eye for b in blocks], axis=1).astype(bf)

    xr = x[:, :, :HALF].reshape(B, S, NCORES, JPC)
    xi = x[:, :, HALF:].reshape(B, S, NCORES, JPC)
    in_maps = []
    for cix in range(NCORES):
        shard = np.empty((ROWS, S), dtype=bf)
        shard[:ROWS // 2] = xr[:, :, cix, :].transpose(0, 2, 1).reshape(
            ROWS // 2, S).astype(bf)
        shard[ROWS // 2:] = xi[:, :, cix, :].transpose(0, 2, 1).reshape(
            ROWS // 2, S).astype(bf)
        in_maps.append({"x": shard, "consts": consts})

    res = run_bass_kernel_spmd(nc, in_maps, core_ids=list(range(NCORES)))
    global last_result
    last_result = res

    out = np.empty((B, S, D), dtype=np.float32)
    o_r = out[:, :, :HALF].reshape(B, S, NCORES, JPC)
    o_i = out[:, :, HALF:].reshape(B, S, NCORES, JPC)
    for cix in range(NCORES):
        oc = np.asarray(res.results[cix]["out"]).astype(np.float32)
        o_r[:, :, cix, :] = oc[:ROWS // 2].reshape(
            B, JPC, S).transpose(0, 2, 1)
        o_i[:, :, cix, :] = oc[ROWS // 2:].reshape(
            B, JPC, S).transpose(0, 2, 1)
    return out
